# revision 19
# baseline (speedup 1.0000x reference)
"""BlockMoE Trainium2 kernel (8 NeuronCores, pure data parallel).

Reference computation (per row b of x [B=65536, 1024]):
  gate:    g = relu(x @ gw1 + gb1); w = softmax(g @ gw2 + gb2)   [B, 64]
  experts: xb = x.reshape(B, 64, 16)
           h1 = relu(xb[:,e] @ ew1[e] + eb1[e])                  [B, 64, 64]
           h2 = relu(h1 @ ew2[e] + eb2[e])                       [B, 64, 32]
           l  = h2 @ ew3[e] + eb3[e]                             [B, 64]
  out:     sum_e w[:,e] * l[:,e]                                 [B, 1]

v3 design (see git history for the fp32r baseline):
  - Shard batch across 8 cores (8192 rows each), replicate params.
  - Tiles of 512 rows processed in PAIRS (tA=2s, tB=2s+1) so that the
    relu+bias ops merge into [128, 2, 512] access patterns spanning two
    PSUM banks: the per-partition bias is identical across the two tiles
    (same expert pair), and merging nearly halves the per-op overhead on
    the ACT/DVE engines, which were the stall source in the bf16 build.
  - L1 + gate run in bf16 (1 col/cycle + fast weight load).  L2 and L3
    run in fp8e4m3 with perf_mode=DoubleRowSwInterleave: one matmul
    contracts two 128-deep K halves (virtual K=256), halving the matmul
    count.  L3 additionally packs BOTH row-tiles into one matmul (rhs
    planes = tile A / tile B; block-structured lhsT planes route A to out
    partitions 64:128 and B to 0:64).  End-to-end rel_l2 error measured
    1.41e-2 on hardware vs the 2e-2 gate (host numpy sim predicts it
    exactly: fp8 quantization of h1/w2/h2/w3 dominates).
  - PSUM budget (8 banks): 2x [128,2,512] h1 psum (4), 1x [128,2,512]
    h2 psum (2), 1 bank logits (lgB at parts 0:64, lgA at 64:128), and
    1 bank "G" time-sliced between the deferred gate tail of pair s-1
    (front half of pair s) and gate-L1 accumulation of pair s (back
    half: all 16 gate matmuls are emitted during chunks 4..7).
  - Softmax-combine folded into exp/sums: out = num/den with
    den = ones^T eg, num = ones^T (eg*l), eg = exp(gate logits).
    den/num matmul outputs are DMA'd to DRAM directly from PSUM.
"""

import sys

sys.path.insert(0, "/opt/trn_rl_repo")

import ml_dtypes
import numpy as np

import concourse.bass as bass
import concourse.mybir as mybir
import concourse.tile as tile
from concourse.bass_utils import run_bass_kernel_spmd

NCORES = 8
B = 65536
FULL = 1024
E = 64
WBLK = 16
HID = 64
GH = 32
BL = B // NCORES  # rows per core
RT = 512  # rows per tile
NT = BL // RT  # tiles per core (16)
NP = NT // 2  # tile pairs per core (8)
NCH = FULL // 128  # x^T chunks per tile (8)

F32 = mybir.dt.float32
BF16 = mybir.dt.bfloat16
FP8 = mybir.dt.float8e4
AF = mybir.ActivationFunctionType
ALU = mybir.AluOpType
PM = mybir.MatmulPerfMode
NPBF16 = np.dtype(ml_dtypes.bfloat16)
NPFP8 = np.dtype(ml_dtypes.float8_e4m3)


def _split_multi_waits(nc, max_waits=1):
    # This walrus build rejects >1 sync-wait on one instruction; move the
    # excess onto fresh EventSemaphore instructions placed just before.
    ctr = 0
    for f in nc.m.functions:
        for blk in f.blocks:
            new_list, changed = [], False
            for inst in blk.instructions:
                si = inst.sync_info
                if si is not None and si.on_wait and len(si.on_wait) > max_waits:
                    waits = list(si.on_wait)
                    excess, keep = waits[:-max_waits], waits[-max_waits:]
                    for w in excess:
                        ev = mybir.InstEventSemaphore(
                            name=f"splitw_{ctr}", ins=[], outs=[]
                        )
                        ctr += 1
                        ev.engine = inst.engine
                        ev.sync_info = mybir.SyncInfo(on_wait=[w], on_update=[])
                        new_list.append(ev)
                    si.on_wait = keep
                    changed = True
                new_list.append(inst)
            if changed:
                blk.instructions = new_list


def _pack_params(gw1, gb1, gw2, gb2, ew1, eb1, ew2, eb2, ew3, eb3):
    """Pack parameters into the SBUF layouts the kernel DMAs verbatim."""
    bf = lambda a: np.ascontiguousarray(a).astype(NPBF16)
    f8 = lambda a: np.ascontiguousarray(a).astype(NPFP8)
    # gate layer 1: lhsT chunks [128, 32] laid out as [128, 8*32]
    gw1s = np.ascontiguousarray(
        gw1.reshape(NCH, 128, GH).transpose(1, 0, 2).reshape(128, NCH * GH)
    )
    # gate layer 2 duplicated on partitions 0:32 (tile B path) and 32:64
    # (tile A path) so both halves of the merged g1s tile can feed it.
    gw2d = np.zeros((64, E), np.float32)
    gw2d[0:32] = gw2
    gw2d[32:64] = gw2
    # L1: pair i = 4c + j covers experts (8c+2j, 8c+2j+1); rhs = x^T chunk c.
    W1 = np.zeros((32, 128, 128), np.float32)
    for i in range(32):
        c, j = divmod(i, 4)
        e0 = 8 * c + 2 * j
        W1[i, 32 * j : 32 * j + 16, 0:64] = ew1[e0]
        W1[i, 32 * j + 16 : 32 * j + 32, 64:128] = ew1[e0 + 1]
    W1s = np.ascontiguousarray(W1.transpose(1, 0, 2).reshape(128, 32 * 128))
    # L2 (fp8 DoubleRow): group q contracts h1 pairs (2q, 2q+1); plane p of
    # the lhsT is the [128,128] block for pair 2q+p.  Covers experts
    # 4q..4q+3 on the 128 output partitions.
    W2 = np.zeros((16, 2, 128, 128), np.float32)
    for q in range(16):
        for c in range(2):
            W2[q, c, 0:64, 64 * c : 64 * c + 32] = ew2[4 * q + 2 * c]
            W2[q, c, 64:128, 64 * c + 32 : 64 * c + 64] = ew2[4 * q + 2 * c + 1]
    W2i = np.zeros((16, 128, 256), np.float32)
    W2i[:, :, 0::2] = W2[:, 0, :, ::-1]
    W2i[:, :, 1::2] = W2[:, 1, :, ::-1]
    W2s = np.ascontiguousarray(W2i.transpose(1, 0, 2).reshape(128, 16 * 256))
    # L3 (fp8 DoubleRow): one matmul per q covers BOTH row-tiles of a
    # pair: rhs plane 0 = tile A's h2, plane 1 = tile B's.  lhsT plane 0
    # routes A to out partitions 64:128, plane 1 routes B to 0:64
    # (matching the egm/lsm half convention).  lhsT col (4q+e) = ew3[4q+e].
    W3 = np.zeros((16, 128, 64), np.float32)
    for q in range(16):
        for e in range(4):
            W3[q, 32 * e : 32 * e + 32, 4 * q + e] = ew3[4 * q + e][:, 0]
    W3d = np.zeros((16, 2, 128, 128), np.float32)
    W3d[:, 0, :, 64:128] = W3
    W3d[:, 1, :, 0:64] = W3
    W3i = np.zeros((16, 128, 256), np.float32)
    W3i[:, :, 0::2] = W3d[:, 0, :, ::-1]
    W3i[:, :, 1::2] = W3d[:, 1, :, ::-1]
    W3s = np.ascontiguousarray(W3i.transpose(1, 0, 2).reshape(128, 16 * 256))
    # biases as per-partition columns
    eb1s = np.ascontiguousarray(eb1.reshape(32, 128).T)  # [128, 32]
    eb2s = np.ascontiguousarray(eb2.reshape(16, 128).T)  # [128, 16]
    # merged-op biases: duplicated per half (B half = 0:64, A half = 64:128)
    gb1d = np.concatenate([gb1, gb1]).reshape(64, 1)
    gb2d = np.concatenate([gb2, gb2]).reshape(128, 1)
    eb3d = np.concatenate([eb3[:, 0], eb3[:, 0]]).reshape(128, 1)
    ones = np.zeros((128, 2), np.float32)
    ones[0:64, 0] = 1.0  # selects tile B's half
    ones[64:128, 1] = 1.0  # selects tile A's half
    return {
        "gw1": bf(gw1s),
        "gw2": bf(gw2d),  # [64, 64]
        "w1": bf(W1s),
        "w2": f8(W2s),
        "w3": f8(W3s),
        "eb1": np.ascontiguousarray(eb1s),
        "eb2": np.ascontiguousarray(eb2s),
        "eb3": np.ascontiguousarray(eb3d),
        "gb1": np.ascontiguousarray(gb1d),
        "gb2": np.ascontiguousarray(gb2d),
        "ones": bf(ones),
    }


def _build_nc(split=True):
    nc = bass.Bass()
    xt = nc.declare_dram_parameter("xt", [NT, NCH, 128, RT], BF16, isOutput=False)
    w1 = nc.declare_dram_parameter("w1", [128, 32 * 128], BF16, isOutput=False)
    w2 = nc.declare_dram_parameter("w2", [128, 16 * 2 * 128], FP8, isOutput=False)
    w3 = nc.declare_dram_parameter("w3", [128, 16 * 2 * 128], FP8, isOutput=False)
    gw1 = nc.declare_dram_parameter("gw1", [128, NCH * GH], BF16, isOutput=False)
    gw2 = nc.declare_dram_parameter("gw2", [64, E], BF16, isOutput=False)
    ones = nc.declare_dram_parameter("ones", [128, 2], BF16, isOutput=False)
    eb1 = nc.declare_dram_parameter("eb1", [128, 32], F32, isOutput=False)
    eb2 = nc.declare_dram_parameter("eb2", [128, 16], F32, isOutput=False)
    eb3 = nc.declare_dram_parameter("eb3", [128, 1], F32, isOutput=False)
    gb1 = nc.declare_dram_parameter("gb1", [64, 1], F32, isOutput=False)
    gb2 = nc.declare_dram_parameter("gb2", [128, 1], F32, isOutput=False)
    # y[t, 0] = denominator row, y[t, 1] = numerator row; host divides.
    y = nc.declare_dram_parameter("y", [NT, 2, RT], F32, isOutput=True)

    with tile.TileContext(nc) as tc:
        with (
            tc.tile_pool(name="consts", bufs=1) as consts,
            tc.tile_pool(name="xp", bufs=34) as xpool,
            tc.tile_pool(name="h1s", bufs=4) as h1pool,
            tc.tile_pool(name="h2s", bufs=3) as h2pool,
            tc.tile_pool(name="gsb", bufs=2) as gpool,
            tc.tile_pool(name="ph1", bufs=2, space="PSUM") as ph1,
            tc.tile_pool(name="ph2", bufs=1, space="PSUM") as ph2,
            tc.tile_pool(name="plg", bufs=1, space="PSUM") as plg,
            tc.tile_pool(name="pgt", bufs=1, space="PSUM") as pgt,
        ):
            # ---- startup: PE warmup dummies first (HAM clock-gate needs
            # ~3.4us of sustained activity), DMAs ordered by first use so
            # the real stream can start as soon as its inputs land.
            gw1t = consts.tile([128, NCH, GH], BF16)
            nc.sync.dma_start(gw1t[:], gw1[:].rearrange("p (c m) -> p c m", c=NCH))
            gb1t = consts.tile([64, 1], F32)
            nc.sync.dma_start(gb1t[:], gb1[:])
            dummy = xpool.tile([128, RT], BF16, tag="xt")
            nc.vector.memset(dummy[:], 0)
            wp = ph1.tile([128, 2, RT], F32, tag="h1p")
            for _ in range(18):
                nc.tensor.matmul(
                    wp[:, 0, :], dummy[:, 0:128], dummy[:], start=True, stop=True
                )

            w1r = w1[:].rearrange("p (s i m) -> p s i m", s=4, i=8)
            w1t = consts.tile([128, 4, 8, 128], BF16)
            nc.sync.dma_start(w1t[:, 0], w1r[:, 0])
            xA, xB = [], []

            def issue_x2(tA, tB, cs, xAl, xBl):
                for c in cs:
                    xa = xpool.tile([128, RT], BF16, tag="xt", name="xa")
                    nc.sync.dma_start(xa[:], xt[tA, c])
                    xAl.append(xa)
                    xb = xpool.tile([128, RT], BF16, tag="xt", name="xb")
                    nc.sync.dma_start(xb[:], xt[tB, c])
                    xBl.append(xb)

            issue_x2(0, 1, range(0, 1), xA, xB)
            eb1t = consts.tile([128, 32], F32)
            nc.sync.dma_start(eb1t[:], eb1[:])
            eb2t = consts.tile([128, 16], F32)
            nc.sync.dma_start(eb2t[:], eb2[:])
            issue_x2(0, 1, range(1, 2), xA, xB)
            nc.sync.dma_start(w1t[:, 1], w1r[:, 1])
            issue_x2(0, 1, range(2, 3), xA, xB)
            w2t = consts.tile([128, 16, 256], FP8)
            nc.sync.dma_start(
                w2t[:], w2[:].rearrange("p (q m) -> p q m", q=16)
            )
            issue_x2(0, 1, range(3, 4), xA, xB)
            nc.sync.dma_start(w1t[:, 2], w1r[:, 2])
            nc.sync.dma_start(w1t[:, 3], w1r[:, 3])
            w3t = consts.tile([128, 16, 256], FP8)
            nc.sync.dma_start(
                w3t[:], w3[:].rearrange("p (q m) -> p q m", q=16)
            )
            eb3t = consts.tile([128, 1], F32)
            nc.sync.dma_start(eb3t[:], eb3[:])
            gw2t = consts.tile([64, E], BF16)
            nc.sync.dma_start(gw2t[:], gw2[:])
            gb2t = consts.tile([128, 1], F32)
            nc.sync.dma_start(gb2t[:], gb2[:])
            onest = consts.tile([128, 2], BF16)
            nc.sync.dma_start(onest[:], ones[:])
            issue_x2(0, 1, range(4, NCH), xA, xB)

            eng_flip = [0]

            def relu2(out_ap, in_ap, bias_ap):
                """Merged bias+relu on alternating engines."""
                if eng_flip[0] == 0:
                    nc.scalar.activation(out_ap, in_ap, AF.Relu, bias=bias_ap)
                else:
                    nc.vector.tensor_scalar(
                        out_ap, in_ap, bias_ap, 0.0, ALU.add, ALU.max
                    )
                eng_flip[0] ^= 1

            # Per-pair rolling state for the software pipeline
            # stages: L1 (duo d) -> relus -> L2DR (d-1) -> h2 relu ->
            #         L3 (d-2) -> logits
            DUOS = 2 * NCH  # 16 duos per pair

            def duo_l1(st, d):
                c, dd = divmod(d, 2)
                i0 = 4 * c + 2 * dd
                xA, xB = st["x"]
                h1d = h1pool.tile([128, 2, 2, RT], FP8, tag="h1d", name="h1d")
                for k, i in enumerate((i0, i0 + 1)):
                    hp = ph1.tile([128, 2, RT], F32, tag="h1p", name="h1p")
                    w1i = w1t[:, i // 8, i % 8, :]
                    nc.tensor.matmul(
                        hp[:, 0, :], w1i, xA[c][:], start=True, stop=True
                    )
                    nc.tensor.matmul(
                        hp[:, 1, :], w1i, xB[c][:], start=True, stop=True
                    )
                    relu2(h1d[:, :, k, :], hp[:, :, :], eb1t[:, i : i + 1])
                return h1d

            def duo_l2(st, d):
                h1d = st["h1d"][d]
                q = d  # duo index == L2 group index
                hp2 = ph2.tile([128, 2, RT], F32, tag="h2p", name="h2p")
                for p in range(2):
                    nc.tensor.matmul(
                        hp2[:, p, :],
                        w2t[:, q],
                        h1d[:, p, :, :],
                        start=True,
                        stop=True,
                        perf_mode=PM.DoubleRowSwInterleave,
                    )
                dd = d % 2
                if dd == 0:
                    st["h2d"] = h2pool.tile([128, 2, 2, RT], FP8, tag="h2d", name="h2d")
                relu2(st["h2d"][:, :, dd, :], hp2[:, :, :], eb2t[:, q : q + 1])
                st["h2for"][d] = st["h2d"]

            def duo_l3(st, d):
                # L3 fp8 DoubleRow: one matmul per duo covers both tiles
                # (rhs planes = tile A / tile B of group q=d); accumulates
                # the full lg bank (A at 64:128, B at 0:64 via the
                # block-structured lhsT planes).
                q = d
                h2d = st["h2for"][d]
                nc.tensor.matmul(
                    st["lg"][:, :],
                    w3t[:, q],
                    h2d[:, :, q % 2, :],
                    start=(q == 0),
                    stop=(q == 15),
                    perf_mode=PM.DoubleRowSwInterleave,
                )

            def gate_l1(st, c):
                xA, xB = st["x"]
                # two chunks' worth of gate matmuls for both tiles, emitted
                # in the back half (c=4..7) while bank G's front-half users
                # (previous pair's tail) are done.
                for cc in (2 * (c - 4), 2 * (c - 4) + 1):
                    nc.tensor.matmul(
                        st["G"][0:GH, :],
                        gw1t[:, cc, :],
                        xA[cc][:],
                        start=(cc == 0),
                        stop=(cc == NCH - 1),
                    )
                    nc.tensor.matmul(
                        st["G"][GH : 2 * GH, :],
                        gw1t[:, cc, :],
                        xB[cc][:],
                        start=(cc == 0),
                        stop=(cc == NCH - 1),
                    )

            def tail(prev, slot):
                """Deferred combine tail of the previous pair, emitted in the
                front-half c slots (0..3) of the current pair while bank G's
                gate-L1 region is idle."""
                if prev is None:
                    return
                G = prev["G"]
                if slot == 0:
                    # g1s relu (both tiles merged; G[0:64] -> g1sm).
                    # g1sm[0:32] = tile A hidden, g1sm[32:64] = tile B.
                    g1sm = gpool.tile([64, RT], BF16, tag="g1s", name="g1sm")
                    nc.vector.tensor_scalar(
                        g1sm[:], G[0:64, :], gb1t[:, 0:1], 0.0, ALU.add, ALU.max
                    )
                    prev["g1sm"] = g1sm
                    # gate L2: tile A logits -> G[64:128], tile B -> G[0:64]
                    nc.tensor.matmul(
                        G[64:128, :],
                        gw2t[0:GH, :],
                        prev["g1sm"][0:GH, :],
                        start=True,
                        stop=True,
                    )
                    nc.tensor.matmul(
                        G[0:64, :],
                        gw2t[GH : 2 * GH, :],
                        prev["g1sm"][GH : 2 * GH, :],
                        start=True,
                        stop=True,
                    )
                elif slot == 1:
                    egm = gpool.tile([128, RT], BF16, tag="egm", name="egm")
                    nc.scalar.activation(egm[:], G[:, :], AF.Exp, bias=gb2t[:, 0:1])
                    prev["egm"] = egm
                    # denominators for both tiles in one matmul:
                    # G[0] = sum(egm[0:64]) = denB, G[1] = denA
                    nc.tensor.matmul(
                        G[0:2, :], onest[:, 0:2], egm[:, :],
                        start=True, stop=True,
                    )
                elif slot == 2:
                    # logits bias (identity) + weight-multiply
                    lsm = gpool.tile([128, RT], BF16, tag="lsm", name="lsm")
                    nc.scalar.activation(
                        lsm[:], prev["lg"][:, :], AF.Identity, bias=eb3t[:, 0:1]
                    )
                    mm = gpool.tile([128, RT], BF16, tag="mm", name="mm")
                    nc.vector.tensor_mul(mm[:], prev["egm"][:], lsm[:])
                    prev["mm"] = mm
                elif slot == 3:
                    mm = prev["mm"]
                    # numerators for both tiles: G[32] = numB, G[33] = numA
                    nc.tensor.matmul(
                        G[32:34, :], onest[:, 0:2], mm[:, :],
                        start=True, stop=True,
                    )
                    tA, tB = prev["tiles"]
                    ot = gpool.tile([128, RT], F32, tag="ot", name="ot")
                    nc.scalar.copy(ot[0:2, :], G[0:2, :])
                    nc.vector.tensor_copy(ot[32:34, :], G[32:34, :])
                    nc.sync.dma_start(y[tA, 0:1, :], ot[1:2, :])
                    nc.sync.dma_start(y[tA, 1:2, :], ot[33:34, :])
                    nc.sync.dma_start(y[tB, 0:1, :], ot[0:1, :])
                    nc.sync.dma_start(y[tB, 1:2, :], ot[32:33, :])

            # Global software pipeline across pair boundaries: at global
            # duo g, emit L1(g), L2(g-1), and L3(g-2 when odd) so the PE
            # never drains between pairs.
            states = {}

            def get_state(s):
                if s not in states:
                    states[s] = {
                        "tiles": (2 * s, 2 * s + 1),
                        "h1d": {},
                        "h2for": {},
                        "x": None,
                        "G": pgt.tile([128, RT], F32, tag="G", name="G"),
                        "lg": plg.tile([128, RT], F32, tag="lg", name="lg"),
                    }
                return states[s]

            get_state(0)["x"] = (xA, xB)
            NDUO = NP * DUOS
            for g in range(NDUO + 2):
                if g < NDUO:
                    s, d = divmod(g, DUOS)
                    st = get_state(s)
                    if d == 0 and s + 1 < NP:
                        # prefetch next pair's x while this pair runs
                        xAn, xBn = [], []
                        issue_x2(2 * s + 2, 2 * s + 3, range(NCH), xAn, xBn)
                        get_state(s + 1)["x"] = (xAn, xBn)
                    if d < 3:
                        # previous pair's deferred combine tail; slots 0-2
                        # must precede this pair's first duo_l3 (lg bank
                        # reuse); slot 3 runs at d=6, before gate_l1
                        # reclaims bank G at d=8.
                        tail(states.get(s - 1), d)
                    elif d == 6:
                        tail(states.get(s - 1), 3)
                    st["h1d"][d] = duo_l1(st, d)
                # L3 and gate matmuls are emitted BEFORE the L2DRs: the
                # L2DR consumes the h1 relus issued one duo ago, so pushing
                # it to the end of the duo gives the ACT/DVE queues ~1us
                # more slack (and delays the ph2 bank-reuse WAR likewise).
                if g >= 2 and g - 2 < NDUO:
                    ps, pd = divmod(g - 2, DUOS)
                    ps = get_state(ps)
                    duo_l3(ps, pd)
                    ps["h2for"].pop(pd)
                if g < NDUO:
                    s, d = divmod(g, DUOS)
                    if d % 2 == 0 and d >= 8:
                        gate_l1(get_state(s), d // 2)
                if g >= 1 and g - 1 < NDUO:
                    ps, pd = divmod(g - 1, DUOS)
                    ps = get_state(ps)
                    duo_l2(ps, pd)
                    ps["h1d"].pop(pd)
            for slot in range(4):
                tail(states[NP - 1], slot)

    if split:
        _split_multi_waits(nc)
    return nc


def _shard_x(x):
    """Per-core blocked transpose: [BL, 1024] -> [NT, NCH, 128, RT] bf16."""
    shards = []
    for s in range(NCORES):
        xs = x[s * BL : (s + 1) * BL]  # [8192, 1024]
        blk = xs.reshape(NT, RT, NCH, 128).transpose(0, 2, 3, 1)
        shards.append(np.ascontiguousarray(blk).astype(NPBF16))
    return shards


def run(inputs, trace=False):
    x = np.asarray(inputs["x"], np.float32)
    params = _pack_params(
        np.asarray(inputs["gw1"], np.float32),
        np.asarray(inputs["gb1"], np.float32),
        np.asarray(inputs["gw2"], np.float32),
        np.asarray(inputs["gb2"], np.float32),
        np.asarray(inputs["ew1"], np.float32),
        np.asarray(inputs["eb1"], np.float32),
        np.asarray(inputs["ew2"], np.float32),
        np.asarray(inputs["eb2"], np.float32),
        np.asarray(inputs["ew3"], np.float32),
        np.asarray(inputs["eb3"], np.float32),
    )
    xshards = _shard_x(x)
    nc = _build_nc()
    in_maps = [{"xt": xshards[s], **params} for s in range(NCORES)]
    res = run_bass_kernel_spmd(nc, in_maps, list(range(NCORES)), trace=trace)
    outs = []
    for s in range(NCORES):
        ys = res.results[s]["y"]  # [NT, 2, RT]: den, num
        outs.append((ys[:, 1, :] / ys[:, 0, :]).reshape(BL, 1))
    return np.concatenate(outs, axis=0), res


def kernel(**inputs) -> np.ndarray:
    out, _ = run(inputs, trace=False)
    return out


# revision 20
# speedup vs baseline: 1.0113x; 1.0113x over previous
"""BlockMoE Trainium2 kernel (8 NeuronCores, pure data parallel).

Reference computation (per row b of x [B=65536, 1024]):
  gate:    g = relu(x @ gw1 + gb1); w = softmax(g @ gw2 + gb2)   [B, 64]
  experts: xb = x.reshape(B, 64, 16)
           h1 = relu(xb[:,e] @ ew1[e] + eb1[e])                  [B, 64, 64]
           h2 = relu(h1 @ ew2[e] + eb2[e])                       [B, 64, 32]
           l  = h2 @ ew3[e] + eb3[e]                             [B, 64]
  out:     sum_e w[:,e] * l[:,e]                                 [B, 1]

v3 design (see git history for the fp32r baseline):
  - Shard batch across 8 cores (8192 rows each), replicate params.
  - Tiles of 512 rows processed in PAIRS (tA=2s, tB=2s+1) so that the
    relu+bias ops merge into [128, 2, 512] access patterns spanning two
    PSUM banks: the per-partition bias is identical across the two tiles
    (same expert pair), and merging nearly halves the per-op overhead on
    the ACT/DVE engines, which were the stall source in the bf16 build.
  - L1 + gate run in bf16 (1 col/cycle + fast weight load).  L2 and L3
    run in fp8e4m3 with perf_mode=DoubleRowSwInterleave: one matmul
    contracts two 128-deep K halves (virtual K=256), halving the matmul
    count.  L3 additionally packs BOTH row-tiles into one matmul (rhs
    planes = tile A / tile B; block-structured lhsT planes route A to out
    partitions 64:128 and B to 0:64).  End-to-end rel_l2 error measured
    1.41e-2 on hardware vs the 2e-2 gate (host numpy sim predicts it
    exactly: fp8 quantization of h1/w2/h2/w3 dominates).
  - PSUM budget (8 banks): 2x [128,2,512] h1 psum (4), 1x [128,2,512]
    h2 psum (2), 1 bank logits (lgB at parts 0:64, lgA at 64:128), and
    1 bank "G" time-sliced between the deferred gate tail of pair s-1
    (front half of pair s) and gate-L1 accumulation of pair s (back
    half: all 16 gate matmuls are emitted during chunks 4..7).
  - Softmax-combine folded into exp/sums: out = num/den with
    den = ones^T eg, num = ones^T (eg*l), eg = exp(gate logits).
    den/num matmul outputs are DMA'd to DRAM directly from PSUM.
"""

import sys

sys.path.insert(0, "/opt/trn_rl_repo")

import ml_dtypes
import numpy as np

import concourse.bass as bass
import concourse.mybir as mybir
import concourse.tile as tile
from concourse.bass_utils import run_bass_kernel_spmd

NCORES = 8
B = 65536
FULL = 1024
E = 64
WBLK = 16
HID = 64
GH = 32
BL = B // NCORES  # rows per core
RT = 512  # rows per tile
NT = BL // RT  # tiles per core (16)
NP = NT // 2  # tile pairs per core (8)
NCH = FULL // 128  # x^T chunks per tile (8)

F32 = mybir.dt.float32
BF16 = mybir.dt.bfloat16
FP8 = mybir.dt.float8e4
AF = mybir.ActivationFunctionType
ALU = mybir.AluOpType
PM = mybir.MatmulPerfMode
NPBF16 = np.dtype(ml_dtypes.bfloat16)
NPFP8 = np.dtype(ml_dtypes.float8_e4m3)


def _split_multi_waits(nc, max_waits=1):
    # This walrus build rejects >1 sync-wait on one instruction; move the
    # excess onto fresh EventSemaphore instructions placed just before.
    ctr = 0
    for f in nc.m.functions:
        for blk in f.blocks:
            new_list, changed = [], False
            for inst in blk.instructions:
                si = inst.sync_info
                if si is not None and si.on_wait and len(si.on_wait) > max_waits:
                    waits = list(si.on_wait)
                    excess, keep = waits[:-max_waits], waits[-max_waits:]
                    for w in excess:
                        ev = mybir.InstEventSemaphore(
                            name=f"splitw_{ctr}", ins=[], outs=[]
                        )
                        ctr += 1
                        ev.engine = inst.engine
                        ev.sync_info = mybir.SyncInfo(on_wait=[w], on_update=[])
                        new_list.append(ev)
                    si.on_wait = keep
                    changed = True
                new_list.append(inst)
            if changed:
                blk.instructions = new_list


def _pack_params(gw1, gb1, gw2, gb2, ew1, eb1, ew2, eb2, ew3, eb3):
    """Pack parameters into the SBUF layouts the kernel DMAs verbatim."""
    bf = lambda a: np.ascontiguousarray(a).astype(NPBF16)
    f8 = lambda a: np.ascontiguousarray(a).astype(NPFP8)
    # gate layer 1: lhsT chunks [128, 32] laid out as [128, 8*32]
    gw1s = np.ascontiguousarray(
        gw1.reshape(NCH, 128, GH).transpose(1, 0, 2).reshape(128, NCH * GH)
    )
    # gate layer 2 as one block matmul for both tiles: g1sm rows 0:32
    # (tile A hidden) route to logit columns 64:128, rows 32:64 (tile B)
    # to columns 0:64.
    gw2d = np.zeros((64, 2 * E), np.float32)
    gw2d[0:32, 64:128] = gw2
    gw2d[32:64, 0:64] = gw2
    # L1: pair i = 4c + j covers experts (8c+2j, 8c+2j+1); rhs = x^T chunk c.
    W1 = np.zeros((32, 128, 128), np.float32)
    for i in range(32):
        c, j = divmod(i, 4)
        e0 = 8 * c + 2 * j
        W1[i, 32 * j : 32 * j + 16, 0:64] = ew1[e0]
        W1[i, 32 * j + 16 : 32 * j + 32, 64:128] = ew1[e0 + 1]
    W1s = np.ascontiguousarray(W1.transpose(1, 0, 2).reshape(128, 32 * 128))
    # L2 (fp8 DoubleRow): group q contracts h1 pairs (2q, 2q+1); plane p of
    # the lhsT is the [128,128] block for pair 2q+p.  Covers experts
    # 4q..4q+3 on the 128 output partitions.
    W2 = np.zeros((16, 2, 128, 128), np.float32)
    for q in range(16):
        for c in range(2):
            W2[q, c, 0:64, 64 * c : 64 * c + 32] = ew2[4 * q + 2 * c]
            W2[q, c, 64:128, 64 * c + 32 : 64 * c + 64] = ew2[4 * q + 2 * c + 1]
    W2i = np.zeros((16, 128, 256), np.float32)
    W2i[:, :, 0::2] = W2[:, 0, :, ::-1]
    W2i[:, :, 1::2] = W2[:, 1, :, ::-1]
    W2s = np.ascontiguousarray(W2i.transpose(1, 0, 2).reshape(128, 16 * 256))
    # L3 (fp8 DoubleRow): one matmul per q covers BOTH row-tiles of a
    # pair: rhs plane 0 = tile A's h2, plane 1 = tile B's.  lhsT plane 0
    # routes A to out partitions 64:128, plane 1 routes B to 0:64
    # (matching the egm/lsm half convention).  lhsT col (4q+e) = ew3[4q+e].
    W3 = np.zeros((16, 128, 64), np.float32)
    for q in range(16):
        for e in range(4):
            W3[q, 32 * e : 32 * e + 32, 4 * q + e] = ew3[4 * q + e][:, 0]
    W3d = np.zeros((16, 2, 128, 128), np.float32)
    W3d[:, 0, :, 64:128] = W3
    W3d[:, 1, :, 0:64] = W3
    W3i = np.zeros((16, 128, 256), np.float32)
    W3i[:, :, 0::2] = W3d[:, 0, :, ::-1]
    W3i[:, :, 1::2] = W3d[:, 1, :, ::-1]
    W3s = np.ascontiguousarray(W3i.transpose(1, 0, 2).reshape(128, 16 * 256))
    # biases as per-partition columns
    eb1s = np.ascontiguousarray(eb1.reshape(32, 128).T)  # [128, 32]
    eb2s = np.ascontiguousarray(eb2.reshape(16, 128).T)  # [128, 16]
    # merged-op biases: duplicated per half (B half = 0:64, A half = 64:128)
    gb1d = np.concatenate([gb1, gb1]).reshape(64, 1)
    gb2d = np.concatenate([gb2, gb2]).reshape(128, 1)
    eb3d = np.concatenate([eb3[:, 0], eb3[:, 0]]).reshape(128, 1)
    ones = np.zeros((128, 2), np.float32)
    ones[0:64, 0] = 1.0  # selects tile B's half
    ones[64:128, 1] = 1.0  # selects tile A's half
    return {
        "gw1": bf(gw1s),
        "gw2": bf(gw2d),  # [64, 64]
        "w1": bf(W1s),
        "w2": f8(W2s),
        "w3": f8(W3s),
        "eb1": np.ascontiguousarray(eb1s),
        "eb2": np.ascontiguousarray(eb2s),
        "eb3": np.ascontiguousarray(eb3d),
        "gb1": np.ascontiguousarray(gb1d),
        "gb2": np.ascontiguousarray(gb2d),
        "ones": bf(ones),
    }


def _build_nc(split=True):
    nc = bass.Bass()
    xt = nc.declare_dram_parameter("xt", [NT, NCH, 128, RT], BF16, isOutput=False)
    w1 = nc.declare_dram_parameter("w1", [128, 32 * 128], BF16, isOutput=False)
    w2 = nc.declare_dram_parameter("w2", [128, 16 * 2 * 128], FP8, isOutput=False)
    w3 = nc.declare_dram_parameter("w3", [128, 16 * 2 * 128], FP8, isOutput=False)
    gw1 = nc.declare_dram_parameter("gw1", [128, NCH * GH], BF16, isOutput=False)
    gw2 = nc.declare_dram_parameter("gw2", [64, 2 * E], BF16, isOutput=False)
    ones = nc.declare_dram_parameter("ones", [128, 2], BF16, isOutput=False)
    eb1 = nc.declare_dram_parameter("eb1", [128, 32], F32, isOutput=False)
    eb2 = nc.declare_dram_parameter("eb2", [128, 16], F32, isOutput=False)
    eb3 = nc.declare_dram_parameter("eb3", [128, 1], F32, isOutput=False)
    gb1 = nc.declare_dram_parameter("gb1", [64, 1], F32, isOutput=False)
    gb2 = nc.declare_dram_parameter("gb2", [128, 1], F32, isOutput=False)
    # y[t, 0] = denominator row, y[t, 1] = numerator row; host divides.
    y = nc.declare_dram_parameter("y", [NT, 2, RT], F32, isOutput=True)

    with tile.TileContext(nc) as tc:
        with (
            tc.tile_pool(name="consts", bufs=1) as consts,
            tc.tile_pool(name="xp", bufs=34) as xpool,
            tc.tile_pool(name="h1s", bufs=4) as h1pool,
            tc.tile_pool(name="h2s", bufs=3) as h2pool,
            tc.tile_pool(name="gsb", bufs=2) as gpool,
            tc.tile_pool(name="ph1", bufs=2, space="PSUM") as ph1,
            tc.tile_pool(name="ph2", bufs=1, space="PSUM") as ph2,
            tc.tile_pool(name="plg", bufs=1, space="PSUM") as plg,
            tc.tile_pool(name="pgt", bufs=1, space="PSUM") as pgt,
        ):
            # ---- startup: PE warmup dummies first (HAM clock-gate needs
            # ~3.4us of sustained activity), DMAs ordered by first use so
            # the real stream can start as soon as its inputs land.
            gw1t = consts.tile([128, NCH, GH], BF16)
            nc.sync.dma_start(gw1t[:], gw1[:].rearrange("p (c m) -> p c m", c=NCH))
            gb1t = consts.tile([64, 1], F32)
            nc.sync.dma_start(gb1t[:], gb1[:])
            dummy = xpool.tile([128, RT], BF16, tag="xt")
            nc.vector.memset(dummy[:], 0)
            wp = ph1.tile([128, 2, RT], F32, tag="h1p")
            for _ in range(18):
                nc.tensor.matmul(
                    wp[:, 0, :], dummy[:, 0:128], dummy[:], start=True, stop=True
                )

            w1r = w1[:].rearrange("p (s i m) -> p s i m", s=4, i=8)
            w1t = consts.tile([128, 4, 8, 128], BF16)
            nc.sync.dma_start(w1t[:, 0], w1r[:, 0])
            xA, xB = [], []

            def issue_x2(tA, tB, cs, xAl, xBl):
                for c in cs:
                    xa = xpool.tile([128, RT], BF16, tag="xt", name="xa")
                    nc.sync.dma_start(xa[:], xt[tA, c])
                    xAl.append(xa)
                    xb = xpool.tile([128, RT], BF16, tag="xt", name="xb")
                    nc.sync.dma_start(xb[:], xt[tB, c])
                    xBl.append(xb)

            issue_x2(0, 1, range(0, 1), xA, xB)
            eb1t = consts.tile([128, 32], F32)
            nc.sync.dma_start(eb1t[:], eb1[:])
            eb2t = consts.tile([128, 16], F32)
            nc.sync.dma_start(eb2t[:], eb2[:])
            issue_x2(0, 1, range(1, 2), xA, xB)
            nc.sync.dma_start(w1t[:, 1], w1r[:, 1])
            issue_x2(0, 1, range(2, 3), xA, xB)
            w2t = consts.tile([128, 16, 256], FP8)
            nc.sync.dma_start(
                w2t[:], w2[:].rearrange("p (q m) -> p q m", q=16)
            )
            issue_x2(0, 1, range(3, 4), xA, xB)
            nc.sync.dma_start(w1t[:, 2], w1r[:, 2])
            nc.sync.dma_start(w1t[:, 3], w1r[:, 3])
            w3t = consts.tile([128, 16, 256], FP8)
            nc.sync.dma_start(
                w3t[:], w3[:].rearrange("p (q m) -> p q m", q=16)
            )
            eb3t = consts.tile([128, 1], F32)
            nc.sync.dma_start(eb3t[:], eb3[:])
            gw2t = consts.tile([64, 2 * E], BF16)
            nc.sync.dma_start(gw2t[:], gw2[:])
            gb2t = consts.tile([128, 1], F32)
            nc.sync.dma_start(gb2t[:], gb2[:])
            onest = consts.tile([128, 2], BF16)
            nc.sync.dma_start(onest[:], ones[:])
            issue_x2(0, 1, range(4, NCH), xA, xB)

            eng_flip = [0]

            def relu2(out_ap, in_ap, bias_ap):
                """Merged bias+relu on alternating engines."""
                if eng_flip[0] == 0:
                    nc.scalar.activation(out_ap, in_ap, AF.Relu, bias=bias_ap)
                else:
                    nc.vector.tensor_scalar(
                        out_ap, in_ap, bias_ap, 0.0, ALU.add, ALU.max
                    )
                eng_flip[0] ^= 1

            # Per-pair rolling state for the software pipeline
            # stages: L1 (duo d) -> relus -> L2DR (d-1) -> h2 relu ->
            #         L3 (d-2) -> logits
            DUOS = 2 * NCH  # 16 duos per pair

            def duo_l1(st, d):
                c, dd = divmod(d, 2)
                i0 = 4 * c + 2 * dd
                xA, xB = st["x"]
                h1d = h1pool.tile([128, 2, 2, RT], FP8, tag="h1d", name="h1d")
                for k, i in enumerate((i0, i0 + 1)):
                    hp = ph1.tile([128, 2, RT], F32, tag="h1p", name="h1p")
                    w1i = w1t[:, i // 8, i % 8, :]
                    nc.tensor.matmul(
                        hp[:, 0, :], w1i, xA[c][:], start=True, stop=True
                    )
                    nc.tensor.matmul(
                        hp[:, 1, :], w1i, xB[c][:], start=True, stop=True
                    )
                    relu2(h1d[:, :, k, :], hp[:, :, :], eb1t[:, i : i + 1])
                return h1d

            def duo_l2(st, d):
                h1d = st["h1d"][d]
                q = d  # duo index == L2 group index
                hp2 = ph2.tile([128, 2, RT], F32, tag="h2p", name="h2p")
                for p in range(2):
                    nc.tensor.matmul(
                        hp2[:, p, :],
                        w2t[:, q],
                        h1d[:, p, :, :],
                        start=True,
                        stop=True,
                        perf_mode=PM.DoubleRowSwInterleave,
                    )
                dd = d % 2
                if dd == 0:
                    st["h2d"] = h2pool.tile([128, 2, 2, RT], FP8, tag="h2d", name="h2d")
                relu2(st["h2d"][:, :, dd, :], hp2[:, :, :], eb2t[:, q : q + 1])
                st["h2for"][d] = st["h2d"]

            def duo_l3(st, d):
                # L3 fp8 DoubleRow: one matmul per duo covers both tiles
                # (rhs planes = tile A / tile B of group q=d); accumulates
                # the full lg bank (A at 64:128, B at 0:64 via the
                # block-structured lhsT planes).
                q = d
                h2d = st["h2for"][d]
                nc.tensor.matmul(
                    st["lg"][:, :],
                    w3t[:, q],
                    h2d[:, :, q % 2, :],
                    start=(q == 0),
                    stop=(q == 15),
                    perf_mode=PM.DoubleRowSwInterleave,
                )

            def gate_l1(st, c):
                xA, xB = st["x"]
                # two chunks' worth of gate matmuls for both tiles, emitted
                # in the back half (c=4..7) while bank G's front-half users
                # (previous pair's tail) are done.
                for cc in (2 * (c - 4), 2 * (c - 4) + 1):
                    nc.tensor.matmul(
                        st["G"][0:GH, :],
                        gw1t[:, cc, :],
                        xA[cc][:],
                        start=(cc == 0),
                        stop=(cc == NCH - 1),
                    )
                    nc.tensor.matmul(
                        st["G"][GH : 2 * GH, :],
                        gw1t[:, cc, :],
                        xB[cc][:],
                        start=(cc == 0),
                        stop=(cc == NCH - 1),
                    )

            def tail(prev, slot):
                """Deferred combine tail of the previous pair, emitted in the
                front-half c slots (0..3) of the current pair while bank G's
                gate-L1 region is idle."""
                if prev is None:
                    return
                G = prev["G"]
                if slot == 0:
                    # g1s relu (both tiles merged; G[0:64] -> g1sm).
                    # g1sm[0:32] = tile A hidden, g1sm[32:64] = tile B.
                    g1sm = gpool.tile([64, RT], BF16, tag="g1s", name="g1sm")
                    nc.vector.tensor_scalar(
                        g1sm[:], G[0:64, :], gb1t[:, 0:1], 0.0, ALU.add, ALU.max
                    )
                    prev["g1sm"] = g1sm
                    # gate L2 for both tiles in one block matmul:
                    # out G[64:128] = tile A logits, G[0:64] = tile B
                    nc.tensor.matmul(
                        G[:, :],
                        gw2t[:, :],
                        prev["g1sm"][:, :],
                        start=True,
                        stop=True,
                    )
                elif slot == 1:
                    egm = gpool.tile([128, RT], BF16, tag="egm", name="egm")
                    nc.scalar.activation(egm[:], G[:, :], AF.Exp, bias=gb2t[:, 0:1])
                    prev["egm"] = egm
                    # denominators for both tiles in one matmul:
                    # G[0] = sum(egm[0:64]) = denB, G[1] = denA
                    nc.tensor.matmul(
                        G[0:2, :], onest[:, 0:2], egm[:, :],
                        start=True, stop=True,
                    )
                elif slot == 2:
                    # logits bias (identity) + weight-multiply
                    lsm = gpool.tile([128, RT], BF16, tag="lsm", name="lsm")
                    nc.scalar.activation(
                        lsm[:], prev["lg"][:, :], AF.Identity, bias=eb3t[:, 0:1]
                    )
                    mm = gpool.tile([128, RT], BF16, tag="mm", name="mm")
                    nc.vector.tensor_mul(mm[:], prev["egm"][:], lsm[:])
                    prev["mm"] = mm
                elif slot == 3:
                    mm = prev["mm"]
                    # numerators for both tiles: G[32] = numB, G[33] = numA
                    nc.tensor.matmul(
                        G[32:34, :], onest[:, 0:2], mm[:, :],
                        start=True, stop=True,
                    )
                    tA, tB = prev["tiles"]
                    ot = gpool.tile([128, RT], F32, tag="ot", name="ot")
                    nc.scalar.copy(ot[0:2, :], G[0:2, :])
                    nc.vector.tensor_copy(ot[32:34, :], G[32:34, :])
                    nc.sync.dma_start(y[tA, 0:1, :], ot[1:2, :])
                    nc.sync.dma_start(y[tA, 1:2, :], ot[33:34, :])
                    nc.sync.dma_start(y[tB, 0:1, :], ot[0:1, :])
                    nc.sync.dma_start(y[tB, 1:2, :], ot[32:33, :])

            # Global software pipeline across pair boundaries: at global
            # duo g, emit L1(g), L2(g-1), and L3(g-2 when odd) so the PE
            # never drains between pairs.
            states = {}

            def get_state(s):
                if s not in states:
                    states[s] = {
                        "tiles": (2 * s, 2 * s + 1),
                        "h1d": {},
                        "h2for": {},
                        "x": None,
                        "G": pgt.tile([128, RT], F32, tag="G", name="G"),
                        "lg": plg.tile([128, RT], F32, tag="lg", name="lg"),
                    }
                return states[s]

            get_state(0)["x"] = (xA, xB)
            NDUO = NP * DUOS
            for g in range(NDUO + 2):
                if g < NDUO:
                    s, d = divmod(g, DUOS)
                    st = get_state(s)
                    if d == 0 and s + 1 < NP:
                        # prefetch next pair's x while this pair runs
                        xAn, xBn = [], []
                        issue_x2(2 * s + 2, 2 * s + 3, range(NCH), xAn, xBn)
                        get_state(s + 1)["x"] = (xAn, xBn)
                    if d < 3:
                        # previous pair's deferred combine tail; slots 0-2
                        # must precede this pair's first duo_l3 (lg bank
                        # reuse); slot 3 runs at d=6, before gate_l1
                        # reclaims bank G at d=8.
                        tail(states.get(s - 1), d)
                    elif d == 6:
                        tail(states.get(s - 1), 3)
                    st["h1d"][d] = duo_l1(st, d)
                    if d % 2 == 0 and d >= 8:
                        gate_l1(st, d // 2)
                if g >= 1 and g - 1 < NDUO:
                    ps, pd = divmod(g - 1, DUOS)
                    ps = get_state(ps)
                    duo_l2(ps, pd)
                    ps["h1d"].pop(pd)
                if g >= 2 and g - 2 < NDUO:
                    ps, pd = divmod(g - 2, DUOS)
                    ps = get_state(ps)
                    duo_l3(ps, pd)
                    ps["h2for"].pop(pd)
            for slot in range(4):
                tail(states[NP - 1], slot)

    if split:
        _split_multi_waits(nc)
    return nc


def _shard_x(x):
    """Per-core blocked transpose: [BL, 1024] -> [NT, NCH, 128, RT] bf16."""
    shards = []
    for s in range(NCORES):
        xs = x[s * BL : (s + 1) * BL]  # [8192, 1024]
        blk = xs.reshape(NT, RT, NCH, 128).transpose(0, 2, 3, 1)
        shards.append(np.ascontiguousarray(blk).astype(NPBF16))
    return shards


def run(inputs, trace=False):
    x = np.asarray(inputs["x"], np.float32)
    params = _pack_params(
        np.asarray(inputs["gw1"], np.float32),
        np.asarray(inputs["gb1"], np.float32),
        np.asarray(inputs["gw2"], np.float32),
        np.asarray(inputs["gb2"], np.float32),
        np.asarray(inputs["ew1"], np.float32),
        np.asarray(inputs["eb1"], np.float32),
        np.asarray(inputs["ew2"], np.float32),
        np.asarray(inputs["eb2"], np.float32),
        np.asarray(inputs["ew3"], np.float32),
        np.asarray(inputs["eb3"], np.float32),
    )
    xshards = _shard_x(x)
    nc = _build_nc()
    in_maps = [{"xt": xshards[s], **params} for s in range(NCORES)]
    res = run_bass_kernel_spmd(nc, in_maps, list(range(NCORES)), trace=trace)
    outs = []
    for s in range(NCORES):
        ys = res.results[s]["y"]  # [NT, 2, RT]: den, num
        outs.append((ys[:, 1, :] / ys[:, 0, :]).reshape(BL, 1))
    return np.concatenate(outs, axis=0), res


def kernel(**inputs) -> np.ndarray:
    out, _ = run(inputs, trace=False)
    return out


# revision 21
# speedup vs baseline: 1.0174x; 1.0060x over previous
"""BlockMoE Trainium2 kernel (8 NeuronCores, pure data parallel).

Reference computation (per row b of x [B=65536, 1024]):
  gate:    g = relu(x @ gw1 + gb1); w = softmax(g @ gw2 + gb2)   [B, 64]
  experts: xb = x.reshape(B, 64, 16)
           h1 = relu(xb[:,e] @ ew1[e] + eb1[e])                  [B, 64, 64]
           h2 = relu(h1 @ ew2[e] + eb2[e])                       [B, 64, 32]
           l  = h2 @ ew3[e] + eb3[e]                             [B, 64]
  out:     sum_e w[:,e] * l[:,e]                                 [B, 1]

v3 design (see git history for the fp32r baseline):
  - Shard batch across 8 cores (8192 rows each), replicate params.
  - Tiles of 512 rows processed in PAIRS (tA=2s, tB=2s+1) so that the
    relu+bias ops merge into [128, 2, 512] access patterns spanning two
    PSUM banks: the per-partition bias is identical across the two tiles
    (same expert pair), and merging nearly halves the per-op overhead on
    the ACT/DVE engines, which were the stall source in the bf16 build.
  - L1 + gate run in bf16 (1 col/cycle + fast weight load).  L2 and L3
    run in fp8e4m3 with perf_mode=DoubleRowSwInterleave: one matmul
    contracts two 128-deep K halves (virtual K=256), halving the matmul
    count.  L3 additionally packs BOTH row-tiles into one matmul (rhs
    planes = tile A / tile B; block-structured lhsT planes route A to out
    partitions 64:128 and B to 0:64).  End-to-end rel_l2 error measured
    1.41e-2 on hardware vs the 2e-2 gate (host numpy sim predicts it
    exactly: fp8 quantization of h1/w2/h2/w3 dominates).
  - PSUM budget (8 banks): 2x [128,2,512] h1 psum (4), 1x [128,2,512]
    h2 psum (2), 1 bank logits (lgB at parts 0:64, lgA at 64:128), and
    1 bank "G" time-sliced between the deferred gate tail of pair s-1
    (front half of pair s) and gate-L1 accumulation of pair s (back
    half: all 16 gate matmuls are emitted during chunks 4..7).
  - Softmax-combine folded into exp/sums: out = num/den with
    den = ones^T eg, num = ones^T (eg*l), eg = exp(gate logits).
    den/num matmul outputs are DMA'd to DRAM directly from PSUM.
"""

import sys

sys.path.insert(0, "/opt/trn_rl_repo")

import ml_dtypes
import numpy as np

import concourse.bass as bass
import concourse.mybir as mybir
import concourse.tile as tile
from concourse.bass_utils import run_bass_kernel_spmd

NCORES = 8
B = 65536
FULL = 1024
E = 64
WBLK = 16
HID = 64
GH = 32
BL = B // NCORES  # rows per core
RT = 512  # rows per tile
NT = BL // RT  # tiles per core (16)
NP = NT // 2  # tile pairs per core (8)
NCH = FULL // 128  # x^T chunks per tile (8)

F32 = mybir.dt.float32
BF16 = mybir.dt.bfloat16
FP8 = mybir.dt.float8e4
AF = mybir.ActivationFunctionType
ALU = mybir.AluOpType
PM = mybir.MatmulPerfMode
NPBF16 = np.dtype(ml_dtypes.bfloat16)
NPFP8 = np.dtype(ml_dtypes.float8_e4m3)


def _split_multi_waits(nc, max_waits=1):
    # This walrus build rejects >1 sync-wait on one instruction; move the
    # excess onto fresh EventSemaphore instructions placed just before.
    ctr = 0
    for f in nc.m.functions:
        for blk in f.blocks:
            new_list, changed = [], False
            for inst in blk.instructions:
                si = inst.sync_info
                if si is not None and si.on_wait and len(si.on_wait) > max_waits:
                    waits = list(si.on_wait)
                    excess, keep = waits[:-max_waits], waits[-max_waits:]
                    for w in excess:
                        ev = mybir.InstEventSemaphore(
                            name=f"splitw_{ctr}", ins=[], outs=[]
                        )
                        ctr += 1
                        ev.engine = inst.engine
                        ev.sync_info = mybir.SyncInfo(on_wait=[w], on_update=[])
                        new_list.append(ev)
                    si.on_wait = keep
                    changed = True
                new_list.append(inst)
            if changed:
                blk.instructions = new_list


def _pack_params(gw1, gb1, gw2, gb2, ew1, eb1, ew2, eb2, ew3, eb3):
    """Pack parameters into the SBUF layouts the kernel DMAs verbatim."""
    bf = lambda a: np.ascontiguousarray(a).astype(NPBF16)
    f8 = lambda a: np.ascontiguousarray(a).astype(NPFP8)
    # gate layer 1: lhsT chunks [128, 32] laid out as [128, 8*32]
    gw1s = np.ascontiguousarray(
        gw1.reshape(NCH, 128, GH).transpose(1, 0, 2).reshape(128, NCH * GH)
    )
    # gate layer 2 as one block matmul for both tiles: g1sm rows 0:32
    # (tile A hidden) route to logit columns 64:128, rows 32:64 (tile B)
    # to columns 0:64.
    gw2d = np.zeros((64, 2 * E), np.float32)
    gw2d[0:32, 64:128] = gw2
    gw2d[32:64, 0:64] = gw2
    # L1: pair i = 4c + j covers experts (8c+2j, 8c+2j+1); rhs = x^T chunk c.
    W1 = np.zeros((32, 128, 128), np.float32)
    for i in range(32):
        c, j = divmod(i, 4)
        e0 = 8 * c + 2 * j
        W1[i, 32 * j : 32 * j + 16, 0:64] = ew1[e0]
        W1[i, 32 * j + 16 : 32 * j + 32, 64:128] = ew1[e0 + 1]
    W1s = np.ascontiguousarray(W1.transpose(1, 0, 2).reshape(128, 32 * 128))
    # L2 (fp8 DoubleRow): group q contracts h1 pairs (2q, 2q+1); plane p of
    # the lhsT is the [128,128] block for pair 2q+p.  Covers experts
    # 4q..4q+3 on the 128 output partitions.
    W2 = np.zeros((16, 2, 128, 128), np.float32)
    for q in range(16):
        for c in range(2):
            W2[q, c, 0:64, 64 * c : 64 * c + 32] = ew2[4 * q + 2 * c]
            W2[q, c, 64:128, 64 * c + 32 : 64 * c + 64] = ew2[4 * q + 2 * c + 1]
    W2i = np.zeros((16, 128, 256), np.float32)
    W2i[:, :, 0::2] = W2[:, 0, :, ::-1]
    W2i[:, :, 1::2] = W2[:, 1, :, ::-1]
    W2s = np.ascontiguousarray(W2i.transpose(1, 0, 2).reshape(128, 16 * 256))
    # L3 (fp8 DoubleRow): one matmul per q covers BOTH row-tiles of a
    # pair: rhs plane 0 = tile A's h2, plane 1 = tile B's.  lhsT plane 0
    # routes A to out partitions 64:128, plane 1 routes B to 0:64
    # (matching the egm/lsm half convention).  lhsT col (4q+e) = ew3[4q+e].
    W3 = np.zeros((16, 128, 64), np.float32)
    for q in range(16):
        for e in range(4):
            W3[q, 32 * e : 32 * e + 32, 4 * q + e] = ew3[4 * q + e][:, 0]
    W3d = np.zeros((16, 2, 128, 128), np.float32)
    W3d[:, 0, :, 64:128] = W3
    W3d[:, 1, :, 0:64] = W3
    W3i = np.zeros((16, 128, 256), np.float32)
    W3i[:, :, 0::2] = W3d[:, 0, :, ::-1]
    W3i[:, :, 1::2] = W3d[:, 1, :, ::-1]
    W3s = np.ascontiguousarray(W3i.transpose(1, 0, 2).reshape(128, 16 * 256))
    # biases as per-partition columns
    eb1s = np.ascontiguousarray(eb1.reshape(32, 128).T)  # [128, 32]
    eb2s = np.ascontiguousarray(eb2.reshape(16, 128).T)  # [128, 16]
    # merged-op biases: duplicated per half (B half = 0:64, A half = 64:128)
    gb1d = np.concatenate([gb1, gb1]).reshape(64, 1)
    gb2d = np.concatenate([gb2, gb2]).reshape(128, 1)
    eb3d = np.concatenate([eb3[:, 0], eb3[:, 0]]).reshape(128, 1)
    ones = np.zeros((128, 2), np.float32)
    ones[0:64, 0] = 1.0  # selects tile B's half
    ones[64:128, 1] = 1.0  # selects tile A's half
    return {
        "gw1": bf(gw1s),
        "gw2": bf(gw2d),  # [64, 64]
        "w1": bf(W1s),
        "w2": f8(W2s),
        "w3": f8(W3s),
        "eb1": np.ascontiguousarray(eb1s),
        "eb2": np.ascontiguousarray(eb2s),
        "eb3": np.ascontiguousarray(eb3d),
        "gb1": np.ascontiguousarray(gb1d),
        "gb2": np.ascontiguousarray(gb2d),
        "ones": bf(ones),
    }


def _build_nc(split=True):
    nc = bass.Bass()
    xt = nc.declare_dram_parameter("xt", [NT, NCH, 128, RT], BF16, isOutput=False)
    w1 = nc.declare_dram_parameter("w1", [128, 32 * 128], BF16, isOutput=False)
    w2 = nc.declare_dram_parameter("w2", [128, 16 * 2 * 128], FP8, isOutput=False)
    w3 = nc.declare_dram_parameter("w3", [128, 16 * 2 * 128], FP8, isOutput=False)
    gw1 = nc.declare_dram_parameter("gw1", [128, NCH * GH], BF16, isOutput=False)
    gw2 = nc.declare_dram_parameter("gw2", [64, 2 * E], BF16, isOutput=False)
    ones = nc.declare_dram_parameter("ones", [128, 2], BF16, isOutput=False)
    eb1 = nc.declare_dram_parameter("eb1", [128, 32], F32, isOutput=False)
    eb2 = nc.declare_dram_parameter("eb2", [128, 16], F32, isOutput=False)
    eb3 = nc.declare_dram_parameter("eb3", [128, 1], F32, isOutput=False)
    gb1 = nc.declare_dram_parameter("gb1", [64, 1], F32, isOutput=False)
    gb2 = nc.declare_dram_parameter("gb2", [128, 1], F32, isOutput=False)
    # y[t, 0] = denominator row, y[t, 1] = numerator row; host divides.
    y = nc.declare_dram_parameter("y", [NT, 2, RT], F32, isOutput=True)

    with tile.TileContext(nc) as tc:
        with (
            tc.tile_pool(name="consts", bufs=1) as consts,
            tc.tile_pool(name="xp", bufs=34) as xpool,
            tc.tile_pool(name="h1s", bufs=4) as h1pool,
            tc.tile_pool(name="h2s", bufs=3) as h2pool,
            tc.tile_pool(name="gsb", bufs=2) as gpool,
            tc.tile_pool(name="ph1", bufs=2, space="PSUM") as ph1,
            tc.tile_pool(name="ph2", bufs=1, space="PSUM") as ph2,
            tc.tile_pool(name="plg", bufs=1, space="PSUM") as plg,
            tc.tile_pool(name="pgt", bufs=1, space="PSUM") as pgt,
        ):
            # ---- startup: PE warmup dummies first (HAM clock-gate needs
            # ~3.4us of sustained activity), DMAs ordered by first use so
            # the real stream can start as soon as its inputs land.
            gw1t = consts.tile([128, NCH, GH], BF16)
            nc.sync.dma_start(gw1t[:], gw1[:].rearrange("p (c m) -> p c m", c=NCH))
            gb1t = consts.tile([64, 1], F32)
            nc.sync.dma_start(gb1t[:], gb1[:])
            dummy = xpool.tile([128, RT], BF16, tag="xt")
            nc.vector.memset(dummy[:], 0)
            wp = ph1.tile([128, 2, RT], F32, tag="h1p")
            for _ in range(32):
                nc.tensor.matmul(
                    wp[:, 0, :], dummy[:, 0:128], dummy[:], start=True, stop=True
                )

            w1r = w1[:].rearrange("p (s i m) -> p s i m", s=4, i=8)
            w1t = consts.tile([128, 4, 8, 128], BF16)
            nc.sync.dma_start(w1t[:, 0], w1r[:, 0])
            xA, xB = [], []

            def issue_x2(tA, tB, cs, xAl, xBl):
                for c in cs:
                    xa = xpool.tile([128, RT], BF16, tag="xt", name="xa")
                    nc.sync.dma_start(xa[:], xt[tA, c])
                    xAl.append(xa)
                    xb = xpool.tile([128, RT], BF16, tag="xt", name="xb")
                    nc.sync.dma_start(xb[:], xt[tB, c])
                    xBl.append(xb)

            issue_x2(0, 1, range(0, 1), xA, xB)
            eb1t = consts.tile([128, 32], F32)
            nc.sync.dma_start(eb1t[:], eb1[:])
            eb2t = consts.tile([128, 16], F32)
            nc.sync.dma_start(eb2t[:], eb2[:])
            issue_x2(0, 1, range(1, 2), xA, xB)
            nc.sync.dma_start(w1t[:, 1], w1r[:, 1])
            issue_x2(0, 1, range(2, 3), xA, xB)
            w2t = consts.tile([128, 16, 256], FP8)
            nc.sync.dma_start(
                w2t[:], w2[:].rearrange("p (q m) -> p q m", q=16)
            )
            issue_x2(0, 1, range(3, 4), xA, xB)
            nc.sync.dma_start(w1t[:, 2], w1r[:, 2])
            nc.sync.dma_start(w1t[:, 3], w1r[:, 3])
            w3t = consts.tile([128, 16, 256], FP8)
            nc.sync.dma_start(
                w3t[:], w3[:].rearrange("p (q m) -> p q m", q=16)
            )
            eb3t = consts.tile([128, 1], F32)
            nc.sync.dma_start(eb3t[:], eb3[:])
            gw2t = consts.tile([64, 2 * E], BF16)
            nc.sync.dma_start(gw2t[:], gw2[:])
            gb2t = consts.tile([128, 1], F32)
            nc.sync.dma_start(gb2t[:], gb2[:])
            onest = consts.tile([128, 2], BF16)
            nc.sync.dma_start(onest[:], ones[:])
            issue_x2(0, 1, range(4, NCH), xA, xB)

            eng_flip = [0]

            def relu2(out_ap, in_ap, bias_ap):
                """Merged bias+relu on alternating engines."""
                if eng_flip[0] == 0:
                    nc.scalar.activation(out_ap, in_ap, AF.Relu, bias=bias_ap)
                else:
                    nc.vector.tensor_scalar(
                        out_ap, in_ap, bias_ap, 0.0, ALU.add, ALU.max
                    )
                eng_flip[0] ^= 1

            # Per-pair rolling state for the software pipeline
            # stages: L1 (duo d) -> relus -> L2DR (d-1) -> h2 relu ->
            #         L3 (d-2) -> logits
            DUOS = 2 * NCH  # 16 duos per pair

            def duo_l1(st, d):
                c, dd = divmod(d, 2)
                i0 = 4 * c + 2 * dd
                xA, xB = st["x"]
                h1d = h1pool.tile([128, 2, 2, RT], FP8, tag="h1d", name="h1d")
                for k, i in enumerate((i0, i0 + 1)):
                    hp = ph1.tile([128, 2, RT], F32, tag="h1p", name="h1p")
                    w1i = w1t[:, i // 8, i % 8, :]
                    nc.tensor.matmul(
                        hp[:, 0, :], w1i, xA[c][:], start=True, stop=True
                    )
                    nc.tensor.matmul(
                        hp[:, 1, :], w1i, xB[c][:], start=True, stop=True
                    )
                    relu2(h1d[:, :, k, :], hp[:, :, :], eb1t[:, i : i + 1])
                return h1d

            def duo_l2(st, d):
                h1d = st["h1d"][d]
                q = d  # duo index == L2 group index
                hp2 = ph2.tile([128, 2, RT], F32, tag="h2p", name="h2p")
                for p in range(2):
                    nc.tensor.matmul(
                        hp2[:, p, :],
                        w2t[:, q],
                        h1d[:, p, :, :],
                        start=True,
                        stop=True,
                        perf_mode=PM.DoubleRowSwInterleave,
                    )
                dd = d % 2
                if dd == 0:
                    st["h2d"] = h2pool.tile([128, 2, 2, RT], FP8, tag="h2d", name="h2d")
                relu2(st["h2d"][:, :, dd, :], hp2[:, :, :], eb2t[:, q : q + 1])
                st["h2for"][d] = st["h2d"]

            def duo_l3(st, d):
                # L3 fp8 DoubleRow: one matmul per duo covers both tiles
                # (rhs planes = tile A / tile B of group q=d); accumulates
                # the full lg bank (A at 64:128, B at 0:64 via the
                # block-structured lhsT planes).
                q = d
                h2d = st["h2for"][d]
                nc.tensor.matmul(
                    st["lg"][:, :],
                    w3t[:, q],
                    h2d[:, :, q % 2, :],
                    start=(q == 0),
                    stop=(q == 15),
                    perf_mode=PM.DoubleRowSwInterleave,
                )

            def gate_l1(st, c):
                xA, xB = st["x"]
                # two chunks' worth of gate matmuls for both tiles, emitted
                # in the back half (c=4..7) while bank G's front-half users
                # (previous pair's tail) are done.
                for cc in (2 * (c - 4), 2 * (c - 4) + 1):
                    nc.tensor.matmul(
                        st["G"][0:GH, :],
                        gw1t[:, cc, :],
                        xA[cc][:],
                        start=(cc == 0),
                        stop=(cc == NCH - 1),
                    )
                    nc.tensor.matmul(
                        st["G"][GH : 2 * GH, :],
                        gw1t[:, cc, :],
                        xB[cc][:],
                        start=(cc == 0),
                        stop=(cc == NCH - 1),
                    )

            def tail(prev, slot):
                """Deferred combine tail of the previous pair, emitted in the
                front-half c slots (0..3) of the current pair while bank G's
                gate-L1 region is idle."""
                if prev is None:
                    return
                G = prev["G"]
                if slot == 0:
                    # g1s relu (both tiles merged; G[0:64] -> g1sm).
                    # g1sm[0:32] = tile A hidden, g1sm[32:64] = tile B.
                    g1sm = gpool.tile([64, RT], BF16, tag="g1s", name="g1sm")
                    nc.vector.tensor_scalar(
                        g1sm[:], G[0:64, :], gb1t[:, 0:1], 0.0, ALU.add, ALU.max
                    )
                    prev["g1sm"] = g1sm
                    # gate L2 for both tiles in one block matmul:
                    # out G[64:128] = tile A logits, G[0:64] = tile B
                    nc.tensor.matmul(
                        G[:, :],
                        gw2t[:, :],
                        prev["g1sm"][:, :],
                        start=True,
                        stop=True,
                    )
                elif slot == 1:
                    egm = gpool.tile([128, RT], BF16, tag="egm", name="egm")
                    nc.scalar.activation(egm[:], G[:, :], AF.Exp, bias=gb2t[:, 0:1])
                    prev["egm"] = egm
                    # denominators for both tiles in one matmul:
                    # G[0] = sum(egm[0:64]) = denB, G[1] = denA
                    nc.tensor.matmul(
                        G[0:2, :], onest[:, 0:2], egm[:, :],
                        start=True, stop=True,
                    )
                elif slot == 2:
                    # logits bias (identity) + weight-multiply
                    lsm = gpool.tile([128, RT], BF16, tag="lsm", name="lsm")
                    nc.scalar.activation(
                        lsm[:], prev["lg"][:, :], AF.Identity, bias=eb3t[:, 0:1]
                    )
                    mm = gpool.tile([128, RT], BF16, tag="mm", name="mm")
                    nc.vector.tensor_mul(mm[:], prev["egm"][:], lsm[:])
                    prev["mm"] = mm
                elif slot == 3:
                    mm = prev["mm"]
                    # numerators for both tiles: G[32] = numB, G[33] = numA
                    nc.tensor.matmul(
                        G[32:34, :], onest[:, 0:2], mm[:, :],
                        start=True, stop=True,
                    )
                    tA, tB = prev["tiles"]
                    ot = gpool.tile([128, RT], F32, tag="ot", name="ot")
                    nc.scalar.copy(ot[0:2, :], G[0:2, :])
                    nc.vector.tensor_copy(ot[32:34, :], G[32:34, :])
                    nc.sync.dma_start(y[tA, 0:1, :], ot[1:2, :])
                    nc.sync.dma_start(y[tA, 1:2, :], ot[33:34, :])
                    nc.sync.dma_start(y[tB, 0:1, :], ot[0:1, :])
                    nc.sync.dma_start(y[tB, 1:2, :], ot[32:33, :])

            # Global software pipeline across pair boundaries: at global
            # duo g, emit L1(g), L2(g-1), and L3(g-2 when odd) so the PE
            # never drains between pairs.
            states = {}

            def get_state(s):
                if s not in states:
                    states[s] = {
                        "tiles": (2 * s, 2 * s + 1),
                        "h1d": {},
                        "h2for": {},
                        "x": None,
                        "G": pgt.tile([128, RT], F32, tag="G", name="G"),
                        "lg": plg.tile([128, RT], F32, tag="lg", name="lg"),
                    }
                return states[s]

            get_state(0)["x"] = (xA, xB)
            NDUO = NP * DUOS
            for g in range(NDUO + 2):
                if g < NDUO:
                    s, d = divmod(g, DUOS)
                    st = get_state(s)
                    if d == 0 and s + 1 < NP:
                        # prefetch next pair's x while this pair runs
                        xAn, xBn = [], []
                        issue_x2(2 * s + 2, 2 * s + 3, range(NCH), xAn, xBn)
                        get_state(s + 1)["x"] = (xAn, xBn)
                    if d < 3:
                        # previous pair's deferred combine tail; slots 0-2
                        # must precede this pair's first duo_l3 (lg bank
                        # reuse); slot 3 runs at d=6, before gate_l1
                        # reclaims bank G at d=8.
                        tail(states.get(s - 1), d)
                    elif d == 6:
                        tail(states.get(s - 1), 3)
                    st["h1d"][d] = duo_l1(st, d)
                    if d % 2 == 0 and d >= 8:
                        gate_l1(st, d // 2)
                if g >= 1 and g - 1 < NDUO:
                    ps, pd = divmod(g - 1, DUOS)
                    ps = get_state(ps)
                    duo_l2(ps, pd)
                    ps["h1d"].pop(pd)
                if g >= 2 and g - 2 < NDUO:
                    ps, pd = divmod(g - 2, DUOS)
                    ps = get_state(ps)
                    duo_l3(ps, pd)
                    ps["h2for"].pop(pd)
            for slot in range(4):
                tail(states[NP - 1], slot)

    if split:
        _split_multi_waits(nc)
    return nc


def _shard_x(x):
    """Per-core blocked transpose: [BL, 1024] -> [NT, NCH, 128, RT] bf16."""
    shards = []
    for s in range(NCORES):
        xs = x[s * BL : (s + 1) * BL]  # [8192, 1024]
        blk = xs.reshape(NT, RT, NCH, 128).transpose(0, 2, 3, 1)
        shards.append(np.ascontiguousarray(blk).astype(NPBF16))
    return shards


def run(inputs, trace=False):
    x = np.asarray(inputs["x"], np.float32)
    params = _pack_params(
        np.asarray(inputs["gw1"], np.float32),
        np.asarray(inputs["gb1"], np.float32),
        np.asarray(inputs["gw2"], np.float32),
        np.asarray(inputs["gb2"], np.float32),
        np.asarray(inputs["ew1"], np.float32),
        np.asarray(inputs["eb1"], np.float32),
        np.asarray(inputs["ew2"], np.float32),
        np.asarray(inputs["eb2"], np.float32),
        np.asarray(inputs["ew3"], np.float32),
        np.asarray(inputs["eb3"], np.float32),
    )
    xshards = _shard_x(x)
    nc = _build_nc()
    in_maps = [{"xt": xshards[s], **params} for s in range(NCORES)]
    res = run_bass_kernel_spmd(nc, in_maps, list(range(NCORES)), trace=trace)
    outs = []
    for s in range(NCORES):
        ys = res.results[s]["y"]  # [NT, 2, RT]: den, num
        outs.append((ys[:, 1, :] / ys[:, 0, :]).reshape(BL, 1))
    return np.concatenate(outs, axis=0), res


def kernel(**inputs) -> np.ndarray:
    out, _ = run(inputs, trace=False)
    return out
